# revision 1
# baseline (speedup 1.0000x reference)
"""Trainium2 Bass kernel for causal multi-head attention (B=4,T=1024,C=1024,H=16,D=64).

Sharding: 8 cores = 4 batches x 2 query-row parities (even/odd global rows).
Every core runs the IDENTICAL program; per-core variation (batch slice, row
parity) is carried entirely in the input data (xT slice, xTq gather, causal
masks), so one SPMD module serves all cores with no collectives.

Per-core device program:
  phase 1: qT[h] = Wq_h^T @ xTq, kT[h] = Wk_h^T @ xT (PE, fp32r),
           v = x @ Wv -> vext [keys, 65] per (tblock, head) with a ones column
  phase 2: per head: scoresT[k-block] = kT_blk^T @ qT (keys on partitions),
           exp via ACT (scale=1/sqrt(D)); causal masking = elementwise mul by
           host-supplied 0/1 masks on the two boundary blocks; AV accumulates
           oT[65, 512] = [v|1]^T @ attnT giving both the output and sum-exp;
           normalize via DVE reciprocal + PE rank-1 broadcast. Heads are
           software-pipelined (scores of head h run while AVs of head h-1 and
           normalization of head h-2 complete) to keep the in-order PE fed.
  phase 3: out = concat-heads @ Wo^T + bo (PE, fp32r), DMA out.

DMA: loads are split across both HWDGE rings (SP + ACT) and ordered so the
operands of the first matmuls arrive first.
"""
import sys

sys.path.insert(0, "/opt/trn_rl_repo")
import numpy as np

B, T, C, H, D = 4, 1024, 1024, 16, 64
N_CORES = 8
NCT = C // 128  # 8 contraction tiles
NTT = T // 128  # 8 t/key blocks
NP = H // 2  # 8 head pairs
QR = 512  # query rows per core
# suffix start per key block: q-block j attends key blocks <= 2j+1
STARTS = [0, 0, 128, 128, 256, 256, 384, 384]

_CACHE = {}


def _build():
    import concourse.bacc as bacc
    import concourse.mybir as mybir
    import concourse.tile as tile

    F32 = mybir.dt.float32
    F32R = mybir.dt.float32r
    Exp = mybir.ActivationFunctionType.Exp

    def r(ap):
        return ap.bitcast(F32R)

    nc = bacc.Bacc("TRN2", target_bir_lowering=False, debug=False, num_devices=N_CORES)
    xT_d = nc.declare_dram_parameter("xT", [C, T], F32, isOutput=False)
    xTq_d = nc.declare_dram_parameter("xTq", [C, QR], F32, isOutput=False)
    wq_d = nc.declare_dram_parameter("wq", [C, H * D], F32, isOutput=False)
    wk_d = nc.declare_dram_parameter("wk", [C, H * D], F32, isOutput=False)
    wv_d = nc.declare_dram_parameter("wv", [C, H * D], F32, isOutput=False)
    woT_d = nc.declare_dram_parameter("woT", [H * D, C], F32, isOutput=False)
    bob_d = nc.declare_dram_parameter("bob", [128, C], F32, isOutput=False)
    m0_d = nc.declare_dram_parameter("mask0", [128, 128], F32, isOutput=False)
    m1_d = nc.declare_dram_parameter("mask1", [128, 128], F32, isOutput=False)
    ones_d = nc.declare_dram_parameter("ones", [128, 128], F32, isOutput=False)
    out_d = nc.declare_dram_parameter("out", [QR, C], F32, isOutput=True)

    with tile.TileContext(nc) as tc:
        with tc.tile_pool(name="keep", bufs=1) as keep:
            # persistent tiles
            qT = keep.tile([128, NP, QR], F32)  # 2-head stack on partitions
            kT = keep.tile([128, NP, T], F32)
            vext = keep.tile([128, NTT, H, 65], F32)
            m0 = keep.tile([128, 128], F32)
            m1 = keep.tile([128, 128], F32)
            bob = keep.tile([128, C], F32)
            ones64 = keep.tile([1, 64], F32)
            ones_sb = keep.tile([128, 128], F32)
            nc.sync.dma_start(r(m0[:]), r(m0_d[:]))
            nc.sync.dma_start(r(m1[:]), r(m1_d[:]))
            nc.sync.dma_start(r(ones64[:]), r(ones_d[0:1, 0:64]))
            nc.sync.dma_start(r(ones_sb[:]), r(ones_d[:, :]))
            nc.sync.dma_start(bob[:], bob_d[:])
            nc.vector.tensor_copy(
                r(vext[:, :, :, 64:65]), r(ones_sb[:].rearrange("p (a b) -> p a b", a=NTT)))

            # ---------------- phase 1: projections ----------------
            with (
                tc.tile_pool(name="xp", bufs=1) as xp,
                tc.tile_pool(name="wp", bufs=2) as wp,
                tc.tile_pool(name="ps_wide", bufs=2, space="PSUM") as ps_wide,
                tc.tile_pool(name="ps_q", bufs=2, space="PSUM") as ps_q,
            ):
                xT = xp.tile([128, NCT, T], F32)
                xTq = xp.tile([128, NCT, QR], F32)
                wq = wp.tile([128, NCT, H * D], F32, tag="w")
                wk = wp.tile([128, NCT, H * D], F32, tag="w")
                # ring SP: xTq then xT; ring ACT: wq then wk.  First qT matmul
                # needs (xTq c0, wq c0) which are first in each ring.
                for c in range(NCT):
                    nc.sync.dma_start(r(xTq[:, c, :]), r(xTq_d[c * 128:(c + 1) * 128, :]))
                    nc.scalar.dma_start(r(wq[:, c, :]), r(wq_d[c * 128:(c + 1) * 128, :]))
                for c in range(NCT):
                    nc.sync.dma_start(r(xT[:, c, :]), r(xT_d[c * 128:(c + 1) * 128, :]))
                    nc.scalar.dma_start(r(wk[:, c, :]), r(wk_d[c * 128:(c + 1) * 128, :]))

                # qT: per head pair accumulate over c tiles
                for p in range(NP):
                    psq = ps_q.tile([128, QR], F32)
                    for c in range(NCT):
                        nc.tensor.matmul(
                            psq[:],
                            r(wq[:, c, p * 128:(p + 1) * 128]),
                            r(xTq[:, c, :]),
                            start=(c == 0),
                            stop=(c == NCT - 1),
                        )
                    nc.vector.tensor_copy(r(qT[:, p, :]), psq[:])
                # kT
                for p in range(NP):
                    psk = ps_wide.tile([128, T], F32, tag="wide")
                    for c in range(NCT):
                        lhs = r(wk[:, c, p * 128:(p + 1) * 128])
                        nc.tensor.matmul(psk[:, 0:512], lhs, r(xT[:, c, 0:512]),
                                         start=(c == 0), stop=(c == NCT - 1))
                        nc.tensor.matmul(psk[:, 512:1024], lhs, r(xT[:, c, 512:1024]),
                                         start=(c == 0), stop=(c == NCT - 1))
                    nc.vector.tensor_copy(r(kT[:, p, :]), psk[:])
                # v (natural layout) -> vext; wv reuses a freed w slot
                wv = wp.tile([128, NCT, H * D], F32, tag="w")
                for c in range(NCT):
                    nc.scalar.dma_start(r(wv[:, c, :]), r(wv_d[c * 128:(c + 1) * 128, :]))
                for tt in range(NTT):
                    psv = ps_wide.tile([128, H * D], F32, tag="wide")
                    for c in range(NCT):
                        lhs = r(xT[:, c, tt * 128:(tt + 1) * 128])
                        nc.tensor.matmul(psv[:, 0:512], lhs, r(wv[:, c, 0:512]),
                                         start=(c == 0), stop=(c == NCT - 1))
                        nc.tensor.matmul(psv[:, 512:1024], lhs, r(wv[:, c, 512:1024]),
                                         start=(c == 0), stop=(c == NCT - 1))
                    nc.vector.tensor_copy(r(vext[:, tt, :, 0:64]), psv[:].rearrange("p (h d) -> p h d", h=H))

            # ---------------- phase 2 + 3 ----------------
            with tc.tile_pool(name="keep2", bufs=1) as keep2:
                proj_in = keep2.tile([128, NP, QR], F32)
                woT = keep2.tile([128, NP, C], F32)
                # prefetch Wo^T during attention (SP ring is idle now)
                for p in range(NP):
                    nc.sync.dma_start(r(woT[:, p, :]), r(woT_d[p * 128:(p + 1) * 128, :]))
                with (
                    tc.tile_pool(name="attn", bufs=5) as attnp,
                    tc.tile_pool(name="smalls", bufs=3) as smalls,
                    tc.tile_pool(name="ps_s", bufs=2, space="PSUM") as ps_s,
                    tc.tile_pool(name="ps_o", bufs=2, space="PSUM") as ps_o,
                    tc.tile_pool(name="ps_bc", bufs=2, space="PSUM") as ps_bc,
                ):
                    # software pipeline over heads:
                    #   stage S(h): scores+exp+mask for all 8 key blocks
                    #   stage A(h): AV accumulation (consumes stage S tiles)
                    #   stage N(h): normalize into proj_in
                    tiles = {}  # h -> list of (kb, st, attn tile)
                    oTs = {}  # h -> oT psum

                    def stage_s(h):
                        p, po = h // 2, (h % 2) * 64
                        lst = []
                        for j in range(NTT // 2):  # key-block pair (2j, 2j+1)
                            st = STARTS[2 * j]
                            nm = max(512 - st, 256)
                            sps = ps_s.tile([128, 2, 512], F32, tag="s")
                            for sub in range(2):
                                kb = 2 * j + sub
                                nc.tensor.matmul(
                                    sps[:, sub, 512 - nm:],
                                    r(kT[po:po + 64, p, kb * 128:(kb + 1) * 128]),
                                    r(qT[po:po + 64, p, 512 - nm:]),
                                    start=True,
                                    stop=True,
                                )
                            at = attnp.tile([128, 2, 512], F32, tag="at")
                            # one exp covers both key blocks of the pair
                            nc.scalar.activation(r(at[:, :, st:]), sps[:, :, st:], Exp, scale=0.125)
                            for sub in range(2):
                                msk = m0 if sub == 0 else m1
                                nc.vector.tensor_mul(
                                    r(at[:, sub, j * 128:(j + 1) * 128]),
                                    r(at[:, sub, j * 128:(j + 1) * 128]),
                                    r(msk[:]),
                                )
                            lst.append((j, st, at))
                        tiles[h] = lst

                    def stage_a(h):
                        oT = ps_o.tile([65, QR], F32, tag="o")
                        oTs[h] = oT
                        for j, st, at in tiles.pop(h):
                            for sub in range(2):
                                kb = 2 * j + sub
                                nc.tensor.matmul(
                                    oT[:, st:],
                                    r(vext[:, kb, h, :]),
                                    r(at[:, sub, st:]),
                                    start=(kb == 0),
                                    stop=(kb == NTT - 1),
                                    skip_group_check=True,
                                )

                    def stage_n(h):
                        p, po = h // 2, (h % 2) * 64
                        oT = oTs.pop(h)
                        rec = smalls.tile([1, QR], F32, tag="rec")
                        with nc.allow_low_precision(reason="fp32r relabel of fp32 reciprocal"):
                            nc.vector.reciprocal(r(rec[:]), oT[64:65, :])
                        bc = ps_bc.tile([64, QR], F32, tag="bc")
                        nc.tensor.matmul(bc[:], r(ones64[:]), r(rec[:]), start=True, stop=True)
                        bcs = smalls.tile([64, QR], F32, tag="bcs")
                        nc.vector.tensor_copy(bcs[:], bc[:])
                        nc.vector.tensor_mul(r(proj_in[po:po + 64, p, :]), oT[0:64, :], bcs[:])

                    for h in range(H + 2):
                        if h < H:
                            stage_s(h)
                        if 1 <= h <= H:
                            stage_a(h - 1)
                        if h >= 2:
                            stage_n(h - 2)

                # ---------------- phase 3: output projection ----------------
                with (
                    tc.tile_pool(name="fin", bufs=2) as finp,
                    tc.tile_pool(name="ps_f", bufs=2, space="PSUM") as ps_f,
                ):
                    for m in range(QR // 128):
                        psf = ps_f.tile([128, C], F32)
                        for p in range(NP):
                            lhs = r(proj_in[:, p, m * 128:(m + 1) * 128])
                            nc.tensor.matmul(psf[:, 0:512], lhs, r(woT[:, p, 0:512]),
                                             start=(p == 0), stop=(p == NP - 1))
                            nc.tensor.matmul(psf[:, 512:1024], lhs, r(woT[:, p, 512:1024]),
                                             start=(p == 0), stop=(p == NP - 1))
                        fin = finp.tile([128, C], F32, tag="fin")
                        nc.vector.tensor_add(fin[:], psf[:], bob[:])
                        nc.sync.dma_start(out_d[m * 128:(m + 1) * 128, :], fin[:])

    nc.compile()
    return nc


def get_nc():
    if "nc" not in _CACHE:
        _CACHE["nc"] = _build()
    return _CACHE["nc"]


def make_in_maps(x, Wq, Wk, Wv, Wo, bo):
    x = np.asarray(x, dtype=np.float32)
    wq = np.ascontiguousarray(np.asarray(Wq, np.float32).transpose(1, 0, 2).reshape(C, H * D))
    wk = np.ascontiguousarray(np.asarray(Wk, np.float32).transpose(1, 0, 2).reshape(C, H * D))
    wv = np.ascontiguousarray(np.asarray(Wv, np.float32).transpose(1, 0, 2).reshape(C, H * D))
    woT = np.ascontiguousarray(np.asarray(Wo, np.float32).T)
    bob = np.ascontiguousarray(np.broadcast_to(np.asarray(bo, np.float32), (128, C)))
    ones = np.ones((128, 128), np.float32)
    k_ = np.arange(128)[:, None]
    i_ = np.arange(128)[None, :]
    in_maps = []
    for core in range(N_CORES):
        b, par = core // 2, core % 2
        xT = np.ascontiguousarray(x[b].T)
        xTq = np.ascontiguousarray(xT[:, par::2])
        m0 = (k_ <= 2 * i_ + par).astype(np.float32)
        m1 = (k_ <= 2 * i_ + par - 128).astype(np.float32)
        in_maps.append({
            "xT": xT, "xTq": xTq, "wq": wq, "wk": wk, "wv": wv,
            "woT": woT, "bob": bob, "mask0": m0, "mask1": m1, "ones": ones,
        })
    return in_maps


def kernel(x, Wq, Wk, Wv, Wo, bo):
    from concourse.bass_utils import run_bass_kernel_spmd

    nc = get_nc()
    in_maps = make_in_maps(x, Wq, Wk, Wv, Wo, bo)
    res = run_bass_kernel_spmd(nc, in_maps, list(range(N_CORES)))
    out = np.empty((B, T, C), np.float32)
    for core in range(N_CORES):
        b, par = core // 2, core % 2
        out[b, par::2, :] = res.results[core]["out"]
    return out



# revision 3
# speedup vs baseline: 1.0117x; 1.0117x over previous
"""Trainium2 Bass kernel for causal multi-head attention (B=4,T=1024,C=1024,H=16,D=64).

Sharding: 8 cores = 4 batches x 2 query-row parities.  SPMD: every core runs
the identical program; per-core variation is carried in the input data only.

Host-side token permutation: each core receives x[b]^T with columns permuted
to [own-parity tokens | other-parity tokens].  Queries are then simply the
first 512 columns (no separate strided q input), and causality becomes
block-triangular: key level j (128 keys of either parity half) is attended by
queries >= 128j, with a plain (+-diagonal) triangular mask on the boundary
block.  Masks are host data, so parity never appears in the program.

Everything flows in bf16 (halves DMA traffic, removes the fp32r small-matmul
penalty); PSUM accumulation stays fp32.

Per-core program:
  phase 1: qT[pair] = Wq^T @ xT[:, :512] (c-outer over 8 live PSUM chains so
           compute rides the weight-DMA stream), kT = Wk^T @ xT,
           v -> vext [keys, 65] per (tblock, head) with a memset ones column
  phase 2: per head (software-pipelined S/A/N over heads):
           scoresT per key level j = kT_blk^T @ qT[128j:] for both parity
           halves into one [128,2,512] PSUM tile; exp via ACT (scale=1/8,
           bf16 out); boundary masking = one Pool multiply per level with the
           stacked [128,2,128] mask; AV accumulates oT[65,512] = [v|1]^T @
           attnT; normalize via DVE reciprocal + PE rank-1 broadcast.
  phase 3: out = concat-heads @ Wo^T + bo (Pool adds bias), DMA out.

Engines: PE matmuls; ACT exp + kT copies; DVE qT copies, reciprocal,
normalize; Pool vext copies, masks, bias adds; SP/ACT HWDGE rings split so
first-needed tiles (xT c0, wq c0) lead both rings.
"""
import sys

sys.path.insert(0, "/opt/trn_rl_repo")
import numpy as np

B, T, C, H, D = 4, 1024, 1024, 16, 64
N_CORES = 8
NCT = C // 128  # 8 contraction tiles
NTT = T // 128  # 8 key blocks (4 levels x 2 parity halves)
NP = H // 2  # 8 head pairs
QR = 512  # query rows per core
NL = 4  # key levels; level j holds key blocks j (own parity) and j+4 (other)

_CACHE = {}


def _build():
    import concourse.bacc as bacc
    import concourse.mybir as mybir
    import concourse.tile as tile

    F32 = mybir.dt.float32
    BF16 = mybir.dt.bfloat16
    F32R = mybir.dt.float32r
    Exp = mybir.ActivationFunctionType.Exp
    Copy = mybir.ActivationFunctionType.Copy

    def r(ap):
        return ap.bitcast(F32R)

    nc = bacc.Bacc("TRN2", target_bir_lowering=False, debug=False, num_devices=N_CORES)
    xT_d = nc.declare_dram_parameter("xT", [C, T], BF16, isOutput=False)
    wq_d = nc.declare_dram_parameter("wq", [C, H * D], BF16, isOutput=False)
    wk_d = nc.declare_dram_parameter("wk", [C, H * D], BF16, isOutput=False)
    wv_d = nc.declare_dram_parameter("wv", [C, H * D], BF16, isOutput=False)
    woT_d = nc.declare_dram_parameter("woT", [H * D, C], BF16, isOutput=False)
    bob_d = nc.declare_dram_parameter("bob", [128, C], F32, isOutput=False)
    m2_d = nc.declare_dram_parameter("m2", [128, 2, 128], BF16, isOutput=False)
    ones_d = nc.declare_dram_parameter("ones", [1, 64], F32, isOutput=False)
    out_d = nc.declare_dram_parameter("out", [QR, C], F32, isOutput=True)

    with tile.TileContext(nc) as tc:
        with tc.tile_pool(name="keep", bufs=1) as keep:
            xT = keep.tile([128, NCT, T], BF16)
            qT = keep.tile([128, NP, QR], BF16)
            kT = keep.tile([128, NP, T], BF16)
            vext = keep.tile([128, NTT, H, 65], BF16)
            m2 = keep.tile([128, 2, 128], BF16)
            ones64 = keep.tile([1, 64], F32)
            bob = keep.tile([128, C], F32)
            woT = keep.tile([128, NP, C], BF16)
            proj_in = keep.tile([128, NP, QR], BF16)

            # SP ring: xT tiles (first-needed), then small mask/ones tiles.
            for c in range(NCT):
                nc.sync.dma_start(xT[:, c, :], xT_d[c * 128:(c + 1) * 128, :])
            nc.sync.dma_start(m2[:], m2_d[:])
            nc.sync.dma_start(ones64[:], ones_d[:])
            # ACT ring: weights in consumption order, then phase-2/3 tiles.
            with tc.tile_pool(name="wp", bufs=1) as wp:
                wq = wp.tile([128, NCT, H * D], BF16)
                wk = wp.tile([128, NCT, H * D], BF16)
                wv = wp.tile([128, NCT, H * D], BF16)
                for c in range(NCT):
                    nc.scalar.dma_start(wq[:, c, :], wq_d[c * 128:(c + 1) * 128, :])
                for c in range(NCT):
                    nc.scalar.dma_start(wk[:, c, :], wk_d[c * 128:(c + 1) * 128, :])
                for c in range(NCT):
                    nc.scalar.dma_start(wv[:, c, :], wv_d[c * 128:(c + 1) * 128, :])
                for p in range(NP):
                    nc.scalar.dma_start(woT[:, p, :], woT_d[p * 128:(p + 1) * 128, :])
                nc.scalar.dma_start(bob[:], bob_d[:])

                nc.gpsimd.memset(vext[:, :, :, 64:65], 1.0)

                # ---------------- phase 1: projections ----------------
                # qT: c-outer with 8 live PSUM chains -> compute starts on
                # (xT c0, wq c0) and tracks the DMA stream.
                with tc.tile_pool(name="ps_q8", bufs=8, space="PSUM") as ps_q8:
                    psqs = [
                        ps_q8.tile([128, QR], F32, tag=f"q{p}", name=f"psq{p}", bufs=1)
                        for p in range(NP)
                    ]
                    for c in range(NCT):
                        for p in range(NP):
                            nc.tensor.matmul(
                                psqs[p][:],
                                wq[:, c, p * 128:(p + 1) * 128],
                                xT[:, c, 0:QR],
                                start=(c == 0),
                                stop=(c == NCT - 1),
                            )
                    for p in range(NP):
                        nc.vector.tensor_copy(qT[:, p, :], psqs[p][:])

                with tc.tile_pool(name="ps_wide", bufs=2, space="PSUM") as ps_wide:
                    # kT: pair-outer chains
                    for p in range(NP):
                        psk = ps_wide.tile([128, T], F32, tag="wide")
                        for c in range(NCT):
                            lhs = wk[:, c, p * 128:(p + 1) * 128]
                            nc.tensor.matmul(psk[:, 0:512], lhs, xT[:, c, 0:512],
                                             start=(c == 0), stop=(c == NCT - 1))
                            nc.tensor.matmul(psk[:, 512:1024], lhs, xT[:, c, 512:1024],
                                             start=(c == 0), stop=(c == NCT - 1))
                        nc.scalar.activation(kT[:, p, :], psk[:], Copy)
                    # v (natural layout) -> vext
                    for tt in range(NTT):
                        psv = ps_wide.tile([128, H * D], F32, tag="wide")
                        for c in range(NCT):
                            lhs = xT[:, c, tt * 128:(tt + 1) * 128]
                            nc.tensor.matmul(psv[:, 0:512], lhs, wv[:, c, 0:512],
                                             start=(c == 0), stop=(c == NCT - 1))
                            nc.tensor.matmul(psv[:, 512:1024], lhs, wv[:, c, 512:1024],
                                             start=(c == 0), stop=(c == NCT - 1))
                        nc.gpsimd.tensor_copy(
                            vext[:, tt, :, 0:64], psv[:].rearrange("p (h d) -> p h d", h=H))

            # ---------------- phase 2: attention ----------------
            with (
                tc.tile_pool(name="attn", bufs=5) as attnp,
                tc.tile_pool(name="smalls", bufs=3) as smalls,
                tc.tile_pool(name="ps_s", bufs=2, space="PSUM") as ps_s,
                tc.tile_pool(name="ps_o", bufs=2, space="PSUM") as ps_o,
                tc.tile_pool(name="ps_bc", bufs=2, space="PSUM") as ps_bc,
            ):
                # software pipeline over heads: S(h) scores+exp+mask,
                # A(h) AV accumulation, N(h) normalize.
                tiles = {}  # h -> list of (level, at tile)
                oTs = {}  # h -> oT psum

                def stage_s(h):
                    p, po = h // 2, (h % 2) * 64
                    lst = []
                    for j in range(NL):
                        st = 128 * j
                        sps = ps_s.tile([128, 2, 512], F32, tag="s")
                        for sub in range(2):
                            kb = j + 4 * sub
                            nc.tensor.matmul(
                                sps[:, sub, st:],
                                kT[po:po + 64, p, kb * 128:(kb + 1) * 128],
                                qT[po:po + 64, p, st:],
                                start=True,
                                stop=True,
                            )
                        at = attnp.tile([128, 2, 512], BF16, tag="at")
                        nc.scalar.activation(at[:, :, st:], sps[:, :, st:], Exp, scale=0.125)
                        nc.gpsimd.tensor_mul(
                            at[:, :, st:st + 128], at[:, :, st:st + 128], m2[:])
                        lst.append((j, at))
                    tiles[h] = lst

                def stage_a(h):
                    oT = ps_o.tile([65, QR], F32, tag="o")
                    oTs[h] = oT
                    for j, at in tiles.pop(h):
                        st = 128 * j
                        for sub in range(2):
                            kb = j + 4 * sub
                            nc.tensor.matmul(
                                oT[:, st:],
                                vext[:, kb, h, :],
                                at[:, sub, st:],
                                start=(j == 0 and sub == 0),
                                stop=(j == NL - 1 and sub == 1),
                                skip_group_check=True,
                            )

                def stage_n(h):
                    p, po = h // 2, (h % 2) * 64
                    oT = oTs.pop(h)
                    rec = smalls.tile([1, QR], F32, tag="rec")
                    nc.vector.reciprocal(rec[:], oT[64:65, :])
                    bc = ps_bc.tile([64, QR], F32, tag="bc")
                    nc.tensor.matmul(bc[:], r(ones64[:]), r(rec[:]), start=True, stop=True)
                    with nc.allow_low_precision(reason="attention output rounded to bf16"):
                        nc.vector.tensor_mul(proj_in[po:po + 64, p, :], oT[0:64, :], bc[:])

                for h in range(H + 2):
                    if h < H:
                        stage_s(h)
                    if 1 <= h <= H:
                        stage_a(h - 1)
                    if h >= 2:
                        stage_n(h - 2)

            # ---------------- phase 3: output projection ----------------
            with (
                tc.tile_pool(name="fin", bufs=2) as finp,
                tc.tile_pool(name="ps_f", bufs=2, space="PSUM") as ps_f,
            ):
                for m in range(QR // 128):
                    psf = ps_f.tile([128, C], F32)
                    for p in range(NP):
                        lhs = proj_in[:, p, m * 128:(m + 1) * 128]
                        nc.tensor.matmul(psf[:, 0:512], lhs, woT[:, p, 0:512],
                                         start=(p == 0), stop=(p == NP - 1))
                        nc.tensor.matmul(psf[:, 512:1024], lhs, woT[:, p, 512:1024],
                                         start=(p == 0), stop=(p == NP - 1))
                    fin = finp.tile([128, C], F32, tag="fin")
                    nc.gpsimd.tensor_add(fin[:], psf[:], bob[:])
                    nc.sync.dma_start(out_d[m * 128:(m + 1) * 128, :], fin[:])

    nc.compile()
    return nc


def get_nc():
    if "nc" not in _CACHE:
        _CACHE["nc"] = _build()
    return _CACHE["nc"]


def make_in_maps(x, Wq, Wk, Wv, Wo, bo):
    import ml_dtypes

    BF = ml_dtypes.bfloat16
    x = np.asarray(x, dtype=np.float32)
    wq = np.ascontiguousarray(
        np.asarray(Wq, np.float32).transpose(1, 0, 2).reshape(C, H * D).astype(BF))
    wk = np.ascontiguousarray(
        np.asarray(Wk, np.float32).transpose(1, 0, 2).reshape(C, H * D).astype(BF))
    wv = np.ascontiguousarray(
        np.asarray(Wv, np.float32).transpose(1, 0, 2).reshape(C, H * D).astype(BF))
    woT = np.ascontiguousarray(np.asarray(Wo, np.float32).T.astype(BF))
    bob = np.ascontiguousarray(np.broadcast_to(np.asarray(bo, np.float32), (128, C)))
    ones = np.ones((1, 64), np.float32)
    k_ = np.arange(128)[:, None]
    i_ = np.arange(128)[None, :]
    m_own = (k_ <= i_).astype(BF)  # own-parity boundary block: k <= q
    in_maps = []
    for core in range(N_CORES):
        b, par = core // 2, core % 2
        perm = np.concatenate([np.arange(par, T, 2), np.arange(1 - par, T, 2)])
        xTp = np.ascontiguousarray(x[b].T[:, perm].astype(BF))
        # other-parity keys 2k+(1-par) vs queries 2i+par: strict for par=0
        m_oth = ((k_ < i_) if par == 0 else (k_ <= i_)).astype(BF)
        m2 = np.ascontiguousarray(np.stack([m_own, m_oth], axis=1))
        in_maps.append({
            "xT": xTp, "wq": wq, "wk": wk, "wv": wv,
            "woT": woT, "bob": bob, "m2": m2, "ones": ones,
        })
    return in_maps


def kernel(x, Wq, Wk, Wv, Wo, bo):
    from concourse.bass_utils import run_bass_kernel_spmd

    nc = get_nc()
    in_maps = make_in_maps(x, Wq, Wk, Wv, Wo, bo)
    res = run_bass_kernel_spmd(nc, in_maps, list(range(N_CORES)))
    out = np.empty((B, T, C), np.float32)
    for core in range(N_CORES):
        b, par = core // 2, core % 2
        out[b, par::2, :] = res.results[core]["out"]
    return out


# revision 22
# speedup vs baseline: 1.1811x; 1.1674x over previous
"""Trainium2 Bass kernel for causal multi-head attention (B=4,T=1024,C=1024,H=16,D=64).

Sharding: 8 cores = 4 batches x 2 query-row parities.  SPMD: every core runs
the identical program; per-core variation is carried in the input data only.

Host-side token permutation: each core receives x[b]^T with columns permuted
to [own-parity tokens | other-parity tokens].  Queries are the first 512
columns, and causality becomes block-triangular: key level j (one 128-key
block per parity half) is attended by queries >= 128j, with a plain
triangular mask on the boundary block.  Masks are host data, so parity never
appears in the program.

Everything flows in bf16 (halves DMA, removes the fp32r small-matmul
penalty); PSUM accumulation stays fp32.

The schedule is built around keeping the PE continuously busy (the tensor
engine clocks down after any idle gap):
  - a short warmup of throwaway matmuls on memset tiles covers the initial
    DMA latency, then qT runs c-outer with 8 live PSUM chains so compute
    rides the x/Wq DMA stream;
  - kT pairs 0-3 and v heads 0-7 follow; attention then starts while the
    remaining projection chains (kT pairs 4-7, v heads 8-15) are interleaved
    one piece per head iteration into the attention stream;
  - per head: scoresT per key level = kT_blk^T @ qT[128j:] for both parity
    halves into one [128,2,512] PSUM tile; exp on ACT (scale=1/8, bf16 out);
    boundary mask = one multiply per level (DVE/Pool alternating); AV
    accumulates oT[65,512] = [v|1]^T @ attnT; normalize via DVE reciprocal +
    PE rank-1 broadcast (the bc tiles share the projection chains' PSUM
    ring); output projection at the end with Pool-free DVE bias adds.
"""
import sys

sys.path.insert(0, "/opt/trn_rl_repo")
import numpy as np

B, T, C, H, D = 4, 1024, 1024, 16, 64
N_CORES = 8
NCT = C // 128  # 8 contraction tiles
NTT = T // 128  # 8 key blocks (4 levels x 2 parity halves)
NP = H // 2  # 8 head pairs
QR = 512  # query rows per core
NL = 4  # key levels; level j holds key blocks j (own parity) and j+4 (other)
WARMUP = 12  # throwaway 512-row matmuls to keep PE busy until first DMA lands

_CACHE = {}


def _build():
    import concourse.bacc as bacc
    import concourse.mybir as mybir
    import concourse.tile as tile

    F32 = mybir.dt.float32
    BF16 = mybir.dt.bfloat16
    F32R = mybir.dt.float32r
    Exp = mybir.ActivationFunctionType.Exp
    Copy = mybir.ActivationFunctionType.Copy

    def r(ap):
        return ap.bitcast(F32R)

    nc = bacc.Bacc("TRN2", target_bir_lowering=False, debug=False, num_devices=N_CORES)
    xT_d = nc.declare_dram_parameter("xT", [C, T], BF16, isOutput=False)
    wq_d = nc.declare_dram_parameter("wq", [C, H * D], BF16, isOutput=False)
    wk_d = nc.declare_dram_parameter("wk", [C, H * D], BF16, isOutput=False)
    wv_d = nc.declare_dram_parameter("wv", [C, H * D], BF16, isOutput=False)
    woT_d = nc.declare_dram_parameter("woT", [H * D, C], BF16, isOutput=False)
    bob_d = nc.declare_dram_parameter("bob", [128, C], F32, isOutput=False)
    m2_d = nc.declare_dram_parameter("m2", [128, 2, 128], BF16, isOutput=False)
    ones_d = nc.declare_dram_parameter("ones", [1, 64], F32, isOutput=False)
    out_d = nc.declare_dram_parameter("out", [QR, C], F32, isOutput=True)

    with tile.TileContext(nc) as tc:
        with tc.tile_pool(name="keep", bufs=1) as keep:
            xT = keep.tile([128, NCT, T], BF16)
            qT = keep.tile([128, NP, QR], BF16)
            kT = keep.tile([128, NP, T], BF16)
            vext = keep.tile([128, NTT, H, 65], BF16)
            m2 = keep.tile([128, 2, 128], BF16)
            lhs1 = keep.tile([1, 64], F32)
            wlhs = keep.tile([1, 64], BF16)
            wsrc = keep.tile([1, QR], BF16)
            dact = keep.tile([1, 16], F32)
            ones_sb = keep.tile([128, 128], BF16)
            bob = keep.tile([128, C], F32)
            woT = keep.tile([128, NP, C], BF16)
            proj_in = keep.tile([128, NP, QR], BF16)

            # memset-built constants: no DMA dependency, so the PE warmup and
            # the bc broadcast never wait on a transfer.
            nc.gpsimd.memset(wlhs[:], 1.0)
            nc.gpsimd.memset(wsrc[:], 0.0)
            nc.gpsimd.memset(ones_sb[:], 1.0)
            nc.vector.tensor_copy(
                vext[:, :, :, 64:65], ones_sb[:].rearrange("p (a b) -> p a b", a=NTT))
            # prime the ACT Exp table while the engine is idle
            nc.scalar.activation(dact[:], wsrc[0:1, 0:16], Exp, scale=0.125)

            # SP ring: xT tiles (first-needed), then the mask.
            for c in range(NCT):
                nc.sync.dma_start(xT[:, c, :], xT_d[c * 128:(c + 1) * 128, :])
            nc.sync.dma_start(m2[:], m2_d[:])
            nc.sync.dma_start(r(lhs1[:]), r(ones_d[:]))
            # ACT ring: weights in consumption order, then phase-3 tiles.
            with tc.tile_pool(name="wp", bufs=1) as wp:
                wq = wp.tile([128, NCT, H * D], BF16)
                wk = wp.tile([128, NCT, H * D], BF16)
                wv = wp.tile([128, NCT, H * D], BF16)
                for c in range(NCT):
                    nc.scalar.dma_start(wq[:, c, :], wq_d[c * 128:(c + 1) * 128, :])
                for c in range(NCT):
                    nc.scalar.dma_start(wk[:, c, :], wk_d[c * 128:(c + 1) * 128, :])
                for c in range(NCT):
                    nc.scalar.dma_start(wv[:, c, :], wv_d[c * 128:(c + 1) * 128, :])
                for p in range(NP):
                    nc.scalar.dma_start(woT[:, p, :], woT_d[p * 128:(p + 1) * 128, :])
                nc.scalar.dma_start(bob[:], bob_d[:])

                # ---- phase 1a: PE warmup + qT (c-outer, rides the DMA) ----
                with tc.tile_pool(name="ps_q8", bufs=1, space="PSUM") as ps_q8:
                    psqs = [
                        ps_q8.tile([128, QR], F32, tag=f"q{p}", name=f"psq{p}", bufs=1)
                        for p in range(NP)
                    ]
                    for w in range(WARMUP):
                        nc.tensor.matmul(
                            psqs[w % NP][0:64, :], wlhs[:], wsrc[:],
                            start=True, stop=True,
                        )
                    for c in range(NCT):
                        for p in range(NP):
                            nc.tensor.matmul(
                                psqs[p][:],
                                wq[:, c, p * 128:(p + 1) * 128],
                                xT[:, c, 0:QR],
                                start=(c == 0),
                                stop=(c == NCT - 1),
                            )
                    for p in range(NP):
                        nc.vector.tensor_copy(qT[:, p, :], psqs[p][:])

                # ---- phase 1b: kT pairs 0-3, v heads 0-7 ----
                with tc.tile_pool(name="ps_w", bufs=2, space="PSUM") as ps_w:
                    for p in range(NP // 2):
                        for u in range(2):
                            psk = ps_w.tile([128, QR], F32, tag="w")
                            for c in range(NCT):
                                nc.tensor.matmul(
                                    psk[:],
                                    wk[:, c, p * 128:(p + 1) * 128],
                                    xT[:, c, u * QR:(u + 1) * QR],
                                    start=(c == 0), stop=(c == NCT - 1))
                            nc.scalar.activation(kT[:, p, u * QR:(u + 1) * QR], psk[:], Copy)
                    for tt in range(NTT):
                        psv = ps_w.tile([128, QR], F32, tag="w")
                        for c in range(NCT):
                            nc.tensor.matmul(
                                psv[:],
                                xT[:, c, tt * 128:(tt + 1) * 128],
                                wv[:, c, 0:QR],
                                start=(c == 0), stop=(c == NCT - 1))
                        nc.vector.tensor_copy(
                            vext[:, tt, 0:8, 0:64],
                            psv[:].rearrange("p (h d) -> p h d", h=8))

                # ---- fused attention + remaining projections ----
                with (
                    tc.tile_pool(name="attn", bufs=8) as attnp,
                    tc.tile_pool(name="smalls", bufs=3) as smalls,
                    tc.tile_pool(name="ps_s", bufs=2, space="PSUM") as ps_s,
                    tc.tile_pool(name="ps_o", bufs=2, space="PSUM") as ps_o,
                    tc.tile_pool(name="ps_x", bufs=2, space="PSUM") as ps_x,
                ):
                    # projection pieces fed one-per-head-iteration into the
                    # attention stream.  vB first (needed by AV of head 8 at
                    # iter 9), kT pairs 4-7 spliced to land before their
                    # scores (head 2p at iter 2p).
                    def vb_piece(tt):
                        def run():
                            psv = ps_x.tile([128, QR], F32, tag="x", name="psvb")
                            for c in range(NCT):
                                nc.tensor.matmul(
                                    psv[:],
                                    xT[:, c, tt * 128:(tt + 1) * 128],
                                    wv[:, c, QR:2 * QR],
                                    start=(c == 0), stop=(c == NCT - 1))
                            nc.vector.tensor_copy(
                                vext[:, tt, 8:16, 0:64],
                                psv[:].rearrange("p (h d) -> p h d", h=8))
                        return run

                    def kt_piece(p, u):
                        def run():
                            psk = ps_x.tile([128, QR], F32, tag="x", name="pskb")
                            for c in range(NCT):
                                nc.tensor.matmul(
                                    psk[:],
                                    wk[:, c, p * 128:(p + 1) * 128],
                                    xT[:, c, u * QR:(u + 1) * QR],
                                    start=(c == 0), stop=(c == NCT - 1))
                            nc.scalar.activation(kT[:, p, u * QR:(u + 1) * QR], psk[:], Copy)
                        return run

                    # per-iteration piece lists.  Deadlines: kT pair p must be
                    # fully written before iteration 2p's scores (slot <=
                    # 2p-1); vB before iteration 9's AV of head 8.
                    pieces = [[] for _ in range(H)]
                    pieces[0] = [vb_piece(0), vb_piece(1)]
                    pieces[1] = [vb_piece(2), vb_piece(3)]
                    pieces[2] = [vb_piece(4), vb_piece(5)]
                    pieces[3] = [kt_piece(4, 0), kt_piece(4, 1)]
                    pieces[4] = [vb_piece(6)]
                    pieces[5] = [vb_piece(7)]
                    pieces[6] = [kt_piece(5, 0)]
                    pieces[7] = [kt_piece(5, 1)]
                    pieces[8] = [kt_piece(6, 0)]
                    pieces[9] = [kt_piece(6, 1)]
                    pieces[11] = [kt_piece(7, 0)]
                    pieces[12] = [kt_piece(7, 1)]

                    tiles = {}  # h -> list of (level, at tile)
                    oTs = {}  # h -> oT psum
                    recs = {}  # h -> reciprocal tile

                    def stage_s(h, js):
                        p, po = h // 2, (h % 2) * 64
                        lst = tiles.setdefault(h, [])
                        for j in js:
                            st = 128 * j
                            sps = ps_s.tile([128, 2, 512], F32, tag="s")
                            for sub in range(2):
                                kb = j + 4 * sub
                                nc.tensor.matmul(
                                    sps[:, sub, st:],
                                    kT[po:po + 64, p, kb * 128:(kb + 1) * 128],
                                    qT[po:po + 64, p, st:],
                                    start=True,
                                    stop=True,
                                )
                            at = attnp.tile([128, 2, 512], BF16, tag="at")
                            nc.scalar.activation(at[:, :, st:], sps[:, :, st:], Exp, scale=0.125)
                            eng = nc.vector if j % 2 == 0 else nc.gpsimd
                            eng.tensor_mul(
                                at[:, :, st:st + 128], at[:, :, st:st + 128], m2[:])
                            lst.append((j, at))

                    def stage_a(h):
                        oT = ps_o.tile([65, QR], F32, tag="o")
                        oTs[h] = oT
                        for j, at in tiles.pop(h):
                            st = 128 * j
                            for sub in range(2):
                                kb = j + 4 * sub
                                nc.tensor.matmul(
                                    oT[:, st:],
                                    vext[:, kb, h, :],
                                    at[:, sub, st:],
                                    start=(j == 0 and sub == 0),
                                    stop=(j == NL - 1 and sub == 1),
                                    skip_group_check=True,
                                )
                        rec = smalls.tile([1, QR], F32, tag="rec")
                        with nc.allow_low_precision(reason="fp32r relabel of fp32 reciprocal"):
                            nc.vector.reciprocal(r(rec[:]), oT[64:65, :])
                        recs[h] = rec

                    def stage_n(h):
                        p, po = h // 2, (h % 2) * 64
                        oT = oTs.pop(h)
                        rec = recs.pop(h)
                        bcf = ps_x.tile([128, QR], F32, tag="x", name="bc")
                        bc = bcf[0:64, :]
                        nc.tensor.matmul(bc[:], r(lhs1[:]), r(rec[:]), start=True, stop=True)
                        bcs = smalls.tile([64, QR], F32, tag="bcs")
                        nc.vector.tensor_copy(bcs[:], bc[:])
                        with nc.allow_low_precision(reason="attention output rounded to bf16"):
                            nc.vector.tensor_mul(proj_in[po:po + 64, p, :], oT[0:64, :], bcs[:])

                    for h in range(H + 2):
                        if h < H:
                            stage_s(h, (0, 1))
                        if h < H:
                            for piece in pieces[h]:
                                piece()
                        if h < H:
                            stage_s(h, (2, 3))
                        if 1 <= h <= H:
                            stage_a(h - 1)
                        if h >= 2:
                            stage_n(h - 2)

            # ---------------- output projection ----------------
            with (
                tc.tile_pool(name="fin", bufs=2) as finp,
                tc.tile_pool(name="ps_f", bufs=2, space="PSUM") as ps_f,
            ):
                for m in range(QR // 128):
                    psf = ps_f.tile([128, C], F32)
                    for p in range(NP):
                        lhs = proj_in[:, p, m * 128:(m + 1) * 128]
                        nc.tensor.matmul(psf[:, 0:512], lhs, woT[:, p, 0:512],
                                         start=(p == 0), stop=(p == NP - 1))
                        nc.tensor.matmul(psf[:, 512:1024], lhs, woT[:, p, 512:1024],
                                         start=(p == 0), stop=(p == NP - 1))
                    fin = finp.tile([128, C], F32, tag="fin")
                    nc.vector.tensor_add(fin[:], psf[:], bob[:])
                    nc.sync.dma_start(out_d[m * 128:(m + 1) * 128, :], fin[:])

    nc.compile()
    return nc


def get_nc():
    if "nc" not in _CACHE:
        _CACHE["nc"] = _build()
    return _CACHE["nc"]


def make_in_maps(x, Wq, Wk, Wv, Wo, bo):
    import ml_dtypes

    BF = ml_dtypes.bfloat16
    x = np.asarray(x, dtype=np.float32)
    wq = np.ascontiguousarray(
        np.asarray(Wq, np.float32).transpose(1, 0, 2).reshape(C, H * D).astype(BF))
    wk = np.ascontiguousarray(
        np.asarray(Wk, np.float32).transpose(1, 0, 2).reshape(C, H * D).astype(BF))
    wv = np.ascontiguousarray(
        np.asarray(Wv, np.float32).transpose(1, 0, 2).reshape(C, H * D).astype(BF))
    woT = np.ascontiguousarray(np.asarray(Wo, np.float32).T.astype(BF))
    bob = np.ascontiguousarray(np.broadcast_to(np.asarray(bo, np.float32), (128, C)))
    ones = np.ones((1, 64), np.float32)
    k_ = np.arange(128)[:, None]
    i_ = np.arange(128)[None, :]
    m_own = (k_ <= i_).astype(BF)  # own-parity boundary block: k <= q
    in_maps = []
    for core in range(N_CORES):
        b, par = core // 2, core % 2
        perm = np.concatenate([np.arange(par, T, 2), np.arange(1 - par, T, 2)])
        xTp = np.ascontiguousarray(x[b].T[:, perm].astype(BF))
        # other-parity keys 2k+(1-par) vs queries 2i+par: strict for par=0
        m_oth = ((k_ < i_) if par == 0 else (k_ <= i_)).astype(BF)
        m2 = np.ascontiguousarray(np.stack([m_own, m_oth], axis=1))
        in_maps.append({
            "xT": xTp, "wq": wq, "wk": wk, "wv": wv,
            "woT": woT, "bob": bob, "m2": m2, "ones": ones,
        })
    return in_maps


def kernel(x, Wq, Wk, Wv, Wo, bo):
    from concourse.bass_utils import run_bass_kernel_spmd

    nc = get_nc()
    in_maps = make_in_maps(x, Wq, Wk, Wv, Wo, bo)
    res = run_bass_kernel_spmd(nc, in_maps, list(range(N_CORES)))
    out = np.empty((B, T, C), np.float32)
    for core in range(N_CORES):
        b, par = core // 2, core % 2
        out[b, par::2, :] = res.results[core]["out"]
    return out


# revision 43
# speedup vs baseline: 1.1893x; 1.0069x over previous
"""Trainium2 Bass kernel for causal multi-head attention (B=4,T=1024,C=1024,H=16,D=64).

Sharding: 8 cores = 4 batches x 2 query-row parities.  SPMD: every core runs
the identical program; per-core variation is carried in the input data only.

Host-side token permutation: each core receives x[b]^T with columns permuted
to [own-parity tokens | other-parity tokens].  Queries are the first 512
columns, and causality becomes block-triangular: key level j (one 128-key
block per parity half) is attended by queries >= 128j, with a plain
triangular mask on the boundary block.  Masks are host data, so parity never
appears in the program.

Everything flows in bf16 (halves DMA, removes the fp32r small-matmul
penalty); PSUM accumulation stays fp32.

The schedule is built around keeping the PE continuously busy (the tensor
engine clocks down after any idle gap):
  - a short warmup of throwaway matmuls on memset tiles covers the initial
    DMA latency, then qT runs c-outer with 8 live PSUM chains so compute
    rides the x/Wq DMA stream;
  - kT pairs 0-3 and v heads 0-7 follow; attention then starts while the
    remaining projection chains (kT pairs 4-7, v heads 8-15) are interleaved
    one piece per head iteration into the attention stream;
  - per head: scoresT per key level = kT_blk^T @ qT[128j:] for both parity
    halves into one [128,2,512] PSUM tile; exp on ACT (scale=1/8, bf16 out);
    boundary mask = one multiply per level (DVE/Pool alternating); AV
    accumulates oT[65,512] = [v|1]^T @ attnT; normalize via DVE reciprocal +
    PE rank-1 broadcast (the bc tiles share the projection chains' PSUM
    ring); output projection at the end with Pool-free DVE bias adds.
"""
import sys

sys.path.insert(0, "/opt/trn_rl_repo")
import numpy as np

B, T, C, H, D = 4, 1024, 1024, 16, 64
N_CORES = 8
NCT = C // 128  # 8 contraction tiles
NTT = T // 128  # 8 key blocks (4 levels x 2 parity halves)
NP = H // 2  # 8 head pairs
QR = 512  # query rows per core
NL = 4  # key levels; level j holds key blocks j (own parity) and j+4 (other)
WARMUP = 12  # throwaway 512-row matmuls to keep PE busy until first DMA lands

_CACHE = {}


def _build():
    import concourse.bacc as bacc
    import concourse.mybir as mybir
    import concourse.tile as tile

    F32 = mybir.dt.float32
    BF16 = mybir.dt.bfloat16
    F32R = mybir.dt.float32r
    Exp = mybir.ActivationFunctionType.Exp
    Copy = mybir.ActivationFunctionType.Copy

    def r(ap):
        return ap.bitcast(F32R)

    nc = bacc.Bacc("TRN2", target_bir_lowering=False, debug=False, num_devices=N_CORES)
    xT_d = nc.declare_dram_parameter("xT", [C, T], BF16, isOutput=False)
    wq_d = nc.declare_dram_parameter("wq", [C, H * D], BF16, isOutput=False)
    wk_d = nc.declare_dram_parameter("wk", [C, H * D], BF16, isOutput=False)
    wv_d = nc.declare_dram_parameter("wv", [C, H * D], BF16, isOutput=False)
    woT_d = nc.declare_dram_parameter("woT", [H * D, C], BF16, isOutput=False)
    bo_d = nc.declare_dram_parameter("bo1", [1, C], BF16, isOutput=False)
    m2_d = nc.declare_dram_parameter("m2", [128, 2, 128], BF16, isOutput=False)
    out_d = nc.declare_dram_parameter("out", [QR, C], F32, isOutput=True)
    # DRAM bounce buffer for the per-head 1/sumexp row: written as [1,QR],
    # read back with a stride-0 partition dim to broadcast across 64 rows.
    recscr_d = nc.declare_dram_parameter("recscr", [H, QR], F32, isOutput=True)

    with tile.TileContext(nc) as tc:
        with tc.tile_pool(name="keep", bufs=1) as keep:
            xT = keep.tile([128, NCT, T], BF16)
            qT = keep.tile([128, NP, QR], BF16)
            kT = keep.tile([128, NP, T], BF16)
            vext = keep.tile([128, NTT, H, 65], BF16)
            m2 = keep.tile([128, 2, 128], BF16)
            wlhs = keep.tile([1, 128], BF16)
            wsrc = keep.tile([1, QR], BF16)
            dact = keep.tile([1, 16], F32)
            ones_sb = keep.tile([128, 128], BF16)
            bo1 = keep.tile([1, C], BF16)
            woT = keep.tile([128, NP, C], BF16)
            proj_in = keep.tile([128, NP, QR], BF16)

            # memset-built constants: no DMA dependency, so the PE warmup and
            # the bc broadcast never wait on a transfer.
            nc.vector.memset(wlhs[:], 1.0)
            nc.vector.memset(wsrc[:], 0.0)
            nc.gpsimd.memset(ones_sb[:], 1.0)
            nc.vector.tensor_copy(
                vext[:, :, :, 64:65], ones_sb[:].rearrange("p (a b) -> p a b", a=NTT))
            # prime the ACT Exp table while the engine is idle
            nc.scalar.activation(dact[:], wsrc[0:1, 0:16], Exp, scale=0.125)

            # SP ring: xT tiles (first-needed, 2 c-tiles per copy), then the
            # mask.  Chunked copies halve the per-DMA overhead.
            for a in range(NCT // 2):
                nc.sync.dma_start(
                    xT[:, 2 * a:2 * a + 2, :],
                    xT_d[2 * a * 128:(2 * a + 2) * 128, :].rearrange(
                        "(a p) t -> p a t", p=128))
            nc.sync.dma_start(m2[:], m2_d[:])
            # ACT ring: weights in consumption order, then phase-3 tiles.
            with tc.tile_pool(name="wp", bufs=1) as wp:
                wq = wp.tile([128, NCT, H * D], BF16)
                wk = wp.tile([128, NCT, H * D], BF16)
                wv = wp.tile([128, NCT, H * D], BF16)
                for w_sb, w_d in ((wq, wq_d), (wk, wk_d), (wv, wv_d)):
                    for a in range(NCT // 2):
                        nc.scalar.dma_start(
                            w_sb[:, 2 * a:2 * a + 2, :],
                            w_d[2 * a * 128:(2 * a + 2) * 128, :].rearrange(
                                "(a p) t -> p a t", p=128))
                for a in range(NP // 2):
                    nc.scalar.dma_start(
                        woT[:, 2 * a:2 * a + 2, :],
                        woT_d[2 * a * 128:(2 * a + 2) * 128, :].rearrange(
                            "(a p) t -> p a t", p=128))
                nc.scalar.dma_start(bo1[:], bo_d[:])

                # ---- phase 1a: PE warmup + qT (c-outer, rides the DMA) ----
                with tc.tile_pool(name="ps_q8", bufs=1, space="PSUM") as ps_q8:
                    psqs = [
                        ps_q8.tile([128, QR], F32, tag=f"q{p}", name=f"psq{p}", bufs=1)
                        for p in range(NP)
                    ]
                    for w in range(WARMUP):
                        nc.tensor.matmul(
                            psqs[w % NP][:], wlhs[:], wsrc[:],
                            start=True, stop=True,
                        )
                    for c in range(NCT):
                        for p in range(NP):
                            nc.tensor.matmul(
                                psqs[p][:],
                                wq[:, c, p * 128:(p + 1) * 128],
                                xT[:, c, 0:QR],
                                start=(c == 0),
                                stop=(c == NCT - 1),
                            )
                            if c == NCT - 1:
                                nc.vector.tensor_copy(qT[:, p, :], psqs[p][:])

                # ---- phase 1b: kT pairs 0-3 (c-outer halves, rides the wk
                # DMA stream), then v heads 0-7 ----
                for u in range(2):
                    with tc.tile_pool(name=f"ps_k{u}", bufs=1, space="PSUM") as ps_k4:
                        psks = [
                            ps_k4.tile([128, QR], F32, tag=f"k{p}", name=f"psk{p}", bufs=1)
                            for p in range(NP // 2)
                        ]
                        for c in range(NCT):
                            for p in range(NP // 2):
                                nc.tensor.matmul(
                                    psks[p][:],
                                    wk[:, c, p * 128:(p + 1) * 128],
                                    xT[:, c, u * QR:(u + 1) * QR],
                                    start=(c == 0), stop=(c == NCT - 1))
                        for p in range(NP // 2):
                            nc.scalar.activation(kT[:, p, u * QR:(u + 1) * QR], psks[p][:], Copy)
                with tc.tile_pool(name="ps_w", bufs=2, space="PSUM") as ps_w:
                    for tt in range(NTT):
                        psv = ps_w.tile([128, QR], F32, tag="w")
                        for c in range(NCT):
                            nc.tensor.matmul(
                                psv[:],
                                xT[:, c, tt * 128:(tt + 1) * 128],
                                wv[:, c, 0:QR],
                                start=(c == 0), stop=(c == NCT - 1))
                        nc.vector.tensor_copy(
                            vext[:, tt, 0:8, 0:64],
                            psv[:].rearrange("p (h d) -> p h d", h=8))

                # ---- fused attention + remaining projections ----
                with (
                    tc.tile_pool(name="attn", bufs=8) as attnp,
                    tc.tile_pool(name="smalls", bufs=3) as smalls,
                    tc.tile_pool(name="ps_s", bufs=2, space="PSUM") as ps_s,
                    tc.tile_pool(name="ps_o", bufs=2, space="PSUM") as ps_o,
                    tc.tile_pool(name="ps_x", bufs=2, space="PSUM") as ps_x,
                ):
                    # projection pieces fed one-per-head-iteration into the
                    # attention stream.  vB first (needed by AV of head 8 at
                    # iter 9), kT pairs 4-7 spliced to land before their
                    # scores (head 2p at iter 2p).
                    def vb_piece(tt):
                        def run():
                            psv = ps_x.tile([128, QR], F32, tag="x", name="psvb")
                            for c in range(NCT):
                                nc.tensor.matmul(
                                    psv[:],
                                    xT[:, c, tt * 128:(tt + 1) * 128],
                                    wv[:, c, QR:2 * QR],
                                    start=(c == 0), stop=(c == NCT - 1))
                            nc.vector.tensor_copy(
                                vext[:, tt, 8:16, 0:64],
                                psv[:].rearrange("p (h d) -> p h d", h=8))
                        return run

                    def kt_piece(p, u):
                        def run():
                            psk = ps_x.tile([128, QR], F32, tag="x", name="pskb")
                            for c in range(NCT):
                                nc.tensor.matmul(
                                    psk[:],
                                    wk[:, c, p * 128:(p + 1) * 128],
                                    xT[:, c, u * QR:(u + 1) * QR],
                                    start=(c == 0), stop=(c == NCT - 1))
                            nc.scalar.activation(kT[:, p, u * QR:(u + 1) * QR], psk[:], Copy)
                        return run

                    # per-iteration piece lists.  Deadlines: kT pair p must be
                    # fully written before iteration 2p's scores (slot <=
                    # 2p-1); vB before iteration 9's AV of head 8.
                    pieces = [[] for _ in range(H)]
                    pieces[0] = [vb_piece(0), vb_piece(1)]
                    pieces[1] = [vb_piece(2), vb_piece(3)]
                    pieces[2] = [vb_piece(4)]
                    pieces[3] = [vb_piece(5)]
                    pieces[4] = [vb_piece(6)]
                    pieces[5] = [vb_piece(7)]
                    pieces[6] = [kt_piece(4, 0)]
                    pieces[7] = [kt_piece(4, 1)]
                    pieces[8] = [kt_piece(5, 0)]
                    pieces[9] = [kt_piece(5, 1)]
                    pieces[10] = [kt_piece(6, 0)]
                    pieces[11] = [kt_piece(6, 1)]
                    pieces[12] = [kt_piece(7, 0)]
                    pieces[13] = [kt_piece(7, 1)]

                    tiles = {}  # h -> list of (level, at tile)
                    oTs = {}  # h -> oT psum
                    recbs = {}  # h -> broadcast 1/sumexp tile

                    def stage_s(h, js):
                        p, po = h // 2, (h % 2) * 64
                        lst = tiles.setdefault(h, [])
                        for j in js:
                            st = 128 * j
                            sps = ps_s.tile([128, 2, 512], F32, tag="s")
                            for sub in range(2):
                                kb = j + 4 * sub
                                nc.tensor.matmul(
                                    sps[:, sub, st:],
                                    kT[po:po + 64, p, kb * 128:(kb + 1) * 128],
                                    qT[po:po + 64, p, st:],
                                    start=True,
                                    stop=True,
                                )
                            at = attnp.tile([128, 2, 512], BF16, tag="at")
                            nc.scalar.activation(at[:, :, st:], sps[:, :, st:], Exp, scale=0.125)
                            eng = nc.vector if j % 2 == 0 else nc.gpsimd
                            eng.tensor_mul(
                                at[:, :, st:st + 128], at[:, :, st:st + 128], m2[:])
                            lst.append((j, at))

                    def stage_a(h):
                        oT = ps_o.tile([65, QR], F32, tag="o")
                        for j, at in tiles.pop(h):
                            st = 128 * j
                            for sub in range(2):
                                kb = j + 4 * sub
                                nc.tensor.matmul(
                                    oT[:, st:],
                                    vext[:, kb, h, :],
                                    at[:, sub, st:],
                                    start=(j == 0 and sub == 0),
                                    stop=(j == NL - 1 and sub == 1),
                                    skip_group_check=True,
                                )
                        rec = smalls.tile([1, QR], F32, tag="rec")
                        nc.vector.reciprocal(rec[:], oT[64:65, :])
                        # copy oT out of PSUM immediately so the bank frees
                        # without waiting for the broadcast round trip
                        oTc = smalls.tile([64, QR], BF16, tag="oTc")
                        with nc.allow_low_precision(reason="attention output rounded to bf16"):
                            nc.vector.tensor_copy(oTc[:], oT[0:64, :])
                        oTs[h] = oTc
                        # bounce through DRAM to broadcast across 64 partitions
                        nc.sync.dma_start(recscr_d[h:h + 1, :], rec[:])
                        recb = smalls.tile([64, QR], F32, tag="recb")
                        nc.sync.dma_start(recb[:], recscr_d[h:h + 1, :].partition_broadcast(64))
                        recbs[h] = recb

                    def stage_n(h):
                        p, po = h // 2, (h % 2) * 64
                        oTc = oTs.pop(h)
                        recb = recbs.pop(h)
                        with nc.allow_low_precision(reason="attention output rounded to bf16"):
                            nc.vector.tensor_mul(proj_in[po:po + 64, p, :], oTc[:], recb[:])

                    for h in range(H + 2):
                        if h < H:
                            stage_s(h, (0, 1))
                        if h < H:
                            for piece in pieces[h]:
                                piece()
                        if h < H:
                            stage_s(h, (2, 3))
                        if 1 <= h <= H:
                            stage_a(h - 1)
                        if h >= 2:
                            stage_n(h - 2)

            # ---------------- output projection ----------------
            with (
                tc.tile_pool(name="fin", bufs=2) as finp,
                tc.tile_pool(name="ps_f", bufs=2, space="PSUM") as ps_f,
            ):
                for m in range(QR // 128):
                    psf = ps_f.tile([128, C], F32)
                    for p in range(NP):
                        lhs = proj_in[:, p, m * 128:(m + 1) * 128]
                        nc.tensor.matmul(psf[:, 0:512], lhs, woT[:, p, 0:512],
                                         start=(p == 0), stop=False)
                        nc.tensor.matmul(psf[:, 512:1024], lhs, woT[:, p, 512:1024],
                                         start=(p == 0), stop=False)
                    # bias via rank-1 accumulate: psf += 1 (x) bo
                    nc.tensor.matmul(psf[:, 0:512], wlhs[:], bo1[0:1, 0:512],
                                     start=False, stop=True)
                    nc.tensor.matmul(psf[:, 512:1024], wlhs[:], bo1[0:1, 512:1024],
                                     start=False, stop=True)
                    fin = finp.tile([128, C], F32, tag="fin")
                    # halves on alternating engines so the copies drain 2x
                    for u in range(2):
                        half = slice(u * 512, (u + 1) * 512)
                        if u == 0:
                            nc.scalar.activation(fin[:, half], psf[:, half], Copy)
                        else:
                            nc.vector.tensor_copy(fin[:, half], psf[:, half])
                        nc.sync.dma_start(
                            out_d[m * 128:(m + 1) * 128, half], fin[:, half])

    nc.compile()
    return nc


def get_nc():
    if "nc" not in _CACHE:
        _CACHE["nc"] = _build()
    return _CACHE["nc"]


def make_in_maps(x, Wq, Wk, Wv, Wo, bo):
    import ml_dtypes

    BF = ml_dtypes.bfloat16
    x = np.asarray(x, dtype=np.float32)
    wq = np.ascontiguousarray(
        np.asarray(Wq, np.float32).transpose(1, 0, 2).reshape(C, H * D).astype(BF))
    wk = np.ascontiguousarray(
        np.asarray(Wk, np.float32).transpose(1, 0, 2).reshape(C, H * D).astype(BF))
    wv = np.ascontiguousarray(
        np.asarray(Wv, np.float32).transpose(1, 0, 2).reshape(C, H * D).astype(BF))
    woT = np.ascontiguousarray(np.asarray(Wo, np.float32).T.astype(BF))
    bo1 = np.ascontiguousarray(np.asarray(bo, np.float32).reshape(1, C).astype(BF))
    k_ = np.arange(128)[:, None]
    i_ = np.arange(128)[None, :]
    m_own = (k_ <= i_).astype(BF)  # own-parity boundary block: k <= q
    in_maps = []
    for core in range(N_CORES):
        b, par = core // 2, core % 2
        perm = np.concatenate([np.arange(par, T, 2), np.arange(1 - par, T, 2)])
        xTp = np.ascontiguousarray(x[b].T[:, perm].astype(BF))
        # other-parity keys 2k+(1-par) vs queries 2i+par: strict for par=0
        m_oth = ((k_ < i_) if par == 0 else (k_ <= i_)).astype(BF)
        m2 = np.ascontiguousarray(np.stack([m_own, m_oth], axis=1))
        in_maps.append({
            "xT": xTp, "wq": wq, "wk": wk, "wv": wv,
            "woT": woT, "bo1": bo1, "m2": m2,
        })
    return in_maps


def kernel(x, Wq, Wk, Wv, Wo, bo):
    from concourse.bass_utils import run_bass_kernel_spmd

    nc = get_nc()
    in_maps = make_in_maps(x, Wq, Wk, Wv, Wo, bo)
    res = run_bass_kernel_spmd(nc, in_maps, list(range(N_CORES)))
    out = np.empty((B, T, C), np.float32)
    for core in range(N_CORES):
        b, par = core // 2, core % 2
        out[b, par::2, :] = res.results[core]["out"]
    return out


# revision 62
# speedup vs baseline: 1.2542x; 1.0546x over previous
"""Trainium2 Bass kernel for causal multi-head attention (B=4,T=1024,C=1024,H=16,D=64).

Sharding: 8 cores = 4 batches x 2 query-row parities.  SPMD: every core runs
the identical program; per-core variation is carried in the input data only.

Host-side token permutation: each core receives x[b]^T with columns permuted
to [own-parity tokens | other-parity tokens].  Queries are the first 512
columns, and causality becomes block-triangular: key level j (one 128-key
block per parity half) is attended by queries >= 128j, with a plain
triangular mask on the boundary block.  Masks are host data, so parity never
appears in the program.

Everything flows in bf16 (halves DMA, removes the fp32r small-matmul
penalty); PSUM accumulation stays fp32.

The schedule is built around keeping the PE continuously busy (the tensor
engine clocks down after any idle gap):
  - a short warmup of throwaway matmuls on memset tiles covers the initial
    DMA latency, then qT runs c-outer with 8 live PSUM chains so compute
    rides the x/Wq DMA stream;
  - kT pairs 0-3 and v heads 0-7 follow; attention then starts while the
    remaining projection chains (kT pairs 4-7, v heads 8-15) are interleaved
    one piece per head iteration into the attention stream;
  - per head: scoresT per key level = kT_blk^T @ qT[128j:] for both parity
    halves into one [128,2,512] PSUM tile; exp on ACT (scale=1/8, bf16 out);
    boundary mask = one multiply per level (DVE/Pool alternating); AV
    accumulates oT[65,512] = [v|1]^T @ attnT; normalize via DVE reciprocal +
    PE rank-1 broadcast (the bc tiles share the projection chains' PSUM
    ring); output projection at the end with Pool-free DVE bias adds.
"""
import sys

sys.path.insert(0, "/opt/trn_rl_repo")
import numpy as np

B, T, C, H, D = 4, 1024, 1024, 16, 64
N_CORES = 8
NCT = C // 128  # 8 contraction tiles
NTT = T // 128  # 8 key blocks (4 levels x 2 parity halves)
NP = H // 2  # 8 head pairs
QR = 512  # query rows per core
NL = 4  # key levels; level j holds key blocks j (own parity) and j+4 (other)
WARMUP = 12  # throwaway 512-row matmuls to keep PE busy until first DMA lands

_CACHE = {}


def _build():
    import concourse.bacc as bacc
    import concourse.mybir as mybir
    import concourse.tile as tile

    F32 = mybir.dt.float32
    BF16 = mybir.dt.bfloat16
    F32R = mybir.dt.float32r
    Exp = mybir.ActivationFunctionType.Exp
    Copy = mybir.ActivationFunctionType.Copy

    def r(ap):
        return ap.bitcast(F32R)

    nc = bacc.Bacc("TRN2", target_bir_lowering=False, debug=False, num_devices=N_CORES)
    xT_d = nc.declare_dram_parameter("xT", [C, T], BF16, isOutput=False)
    wq_d = nc.declare_dram_parameter("wq", [C, H * D], BF16, isOutput=False)
    wk_d = nc.declare_dram_parameter("wk", [C, H * D], BF16, isOutput=False)
    wv_d = nc.declare_dram_parameter("wv", [C, H * D], BF16, isOutput=False)
    woT_d = nc.declare_dram_parameter("woT", [H * D, C], BF16, isOutput=False)
    bo_d = nc.declare_dram_parameter("bo1", [1, C], BF16, isOutput=False)
    m2_d = nc.declare_dram_parameter("m2", [128, 2, 128], BF16, isOutput=False)
    out_d = nc.declare_dram_parameter("out", [QR, C], F32, isOutput=True)
    # DRAM bounce buffer for the per-head 1/sumexp row: written as [1,QR],
    # read back with a stride-0 partition dim to broadcast across 64 rows.
    recscr_d = nc.declare_dram_parameter("recscr", [H, QR], F32, isOutput=True)

    with tile.TileContext(nc) as tc:
        with tc.tile_pool(name="keep", bufs=1) as keep:
            xT = keep.tile([128, NCT, T], BF16)
            qT = keep.tile([128, NP, QR], BF16)
            kT = keep.tile([128, NP, T], BF16)
            vext = keep.tile([128, NTT, H, 65], BF16)
            m2 = keep.tile([128, 2, 128], BF16)
            wlhs = keep.tile([1, 128], BF16)
            ones1f = keep.tile([1, 64], F32)
            ones1s = keep.tile([1, 64], F32)
            wsrc = keep.tile([1, QR], BF16)
            dact = keep.tile([1, 16], F32)
            ones_sb = keep.tile([128, 128], BF16)
            bo1 = keep.tile([1, C], BF16)
            woT = keep.tile([128, NP, C], BF16)
            proj_in = keep.tile([128, NP, QR], BF16)

            # memset-built constants: no DMA dependency, so the PE warmup and
            # the bc broadcast never wait on a transfer.
            nc.vector.memset(wlhs[:], 1.0)
            nc.vector.memset(ones1s[:], 1.0)
            with nc.allow_low_precision(reason="fp32r relabel of fp32 ones"):
                nc.vector.tensor_copy(r(ones1f[:]), ones1s[:])
            nc.gpsimd.memset(wsrc[:], 0.0)
            nc.gpsimd.memset(ones_sb[:], 1.0)
            nc.vector.tensor_copy(
                vext[:, :, :, 64:65], ones_sb[:].rearrange("p (a b) -> p a b", a=NTT))
            # prime the ACT Exp table while the engine is idle
            nc.scalar.activation(dact[:], wsrc[0:1, 0:16], Exp, scale=0.125)

            # SP ring: xT tiles (first-needed, 2 c-tiles per copy), then the
            # mask.  Chunked copies halve the per-DMA overhead.
            for a in range(NCT // 2):
                nc.sync.dma_start(
                    xT[:, 2 * a:2 * a + 2, :],
                    xT_d[2 * a * 128:(2 * a + 2) * 128, :].rearrange(
                        "(a p) t -> p a t", p=128))
            nc.sync.dma_start(m2[:], m2_d[:])
            # ACT ring: weights in consumption order, then phase-3 tiles.
            with tc.tile_pool(name="wp", bufs=1) as wp:
                wq = wp.tile([128, NCT, H * D], BF16)
                wk = wp.tile([128, NCT, H * D], BF16)
                wv = wp.tile([128, NCT, H * D], BF16)
                # wq/wk chunks interleaved so pass 1 (qT-A + kT-u0) can
                # consume both weight streams chunk by chunk
                for a in range(NCT // 2):
                    for w_sb, w_d in ((wq, wq_d), (wk, wk_d)):
                        nc.scalar.dma_start(
                            w_sb[:, 2 * a:2 * a + 2, :],
                            w_d[2 * a * 128:(2 * a + 2) * 128, :].rearrange(
                                "(a p) t -> p a t", p=128))
                for a in range(NCT // 2):
                    nc.scalar.dma_start(
                        wv[:, 2 * a:2 * a + 2, :],
                        wv_d[2 * a * 128:(2 * a + 2) * 128, :].rearrange(
                            "(a p) t -> p a t", p=128))
                for a in range(NP // 2):
                    nc.scalar.dma_start(
                        woT[:, 2 * a:2 * a + 2, :],
                        woT_d[2 * a * 128:(2 * a + 2) * 128, :].rearrange(
                            "(a p) t -> p a t", p=128))
                nc.scalar.dma_start(bo1[:], bo_d[:])

                # ---- phase 1a/1b: PE warmup, then qT and kT pairs 0-3 as
                # c-outer 4-chain passes ping-ponging two PSUM bank groups;
                # each pass's copies drain while the next pass computes. ----
                with tc.tile_pool(name="ps8", bufs=1, space="PSUM") as ps8p:
                    ps8 = [
                        ps8p.tile([128, QR], F32, tag=f"g{i}", name=f"ps8_{i}", bufs=1)
                        for i in range(8)
                    ]
                    for w in range(WARMUP):
                        nc.tensor.matmul(
                            ps8[4 + w % 4][:], wlhs[:], wsrc[:],
                            start=True, stop=True,
                        )

                    def couter_pass(jobs):
                        # jobs: list of (bank, w_sb, pair, cols, dst, copy_act)
                        for c in range(NCT):
                            for i, (w_sb, p, cols, dst, copy_act) in enumerate(jobs):
                                nc.tensor.matmul(
                                    ps8[i][:],
                                    w_sb[:, c, p * 128:(p + 1) * 128],
                                    xT[:, c, cols],
                                    start=(c == 0),
                                    stop=(c == NCT - 1),
                                )
                                if c == NCT - 1:
                                    if copy_act:
                                        nc.scalar.activation(dst, ps8[i][:], Copy)
                                    else:
                                        nc.vector.tensor_copy(dst, ps8[i][:])

                    colsq = slice(0, QR)
                    cols0, cols1 = slice(0, QR), slice(QR, 2 * QR)
                    couter_pass(
                        [(wq, p, colsq, qT[:, p, :], False) for p in range(0, 4)]
                        + [(wk, p, cols0, kT[:, p, cols0], True) for p in range(0, 4)])
                    couter_pass(
                        [(wq, p, colsq, qT[:, p, :], False) for p in range(4, 8)]
                        + [(wk, p, cols1, kT[:, p, cols1], True) for p in range(0, 4)])
                    # vA reuses ps8 group-0 tiles round-robin: no pool
                    # transition, so no wait on the full pool release
                    for tt in range(NTT):
                        psv = ps8[tt % 4]
                        for c in range(NCT):
                            nc.tensor.matmul(
                                psv[:],
                                xT[:, c, tt * 128:(tt + 1) * 128],
                                wv[:, c, 0:QR],
                                start=(c == 0), stop=(c == NCT - 1))
                        nc.vector.tensor_copy(
                            vext[:, tt, 0:8, 0:64],
                            psv[:].rearrange("p (h d) -> p h d", h=8))

                # ---- fused attention + remaining projections ----
                with (
                    tc.tile_pool(name="attn", bufs=8) as attnp,
                    tc.tile_pool(name="smalls", bufs=3) as smalls,
                    tc.tile_pool(name="ps_s", bufs=2, space="PSUM") as ps_s,
                    tc.tile_pool(name="ps_o", bufs=2, space="PSUM") as ps_o,
                    tc.tile_pool(name="ps_x", bufs=2, space="PSUM") as ps_x,
                ):
                    # projection pieces fed one-per-head-iteration into the
                    # attention stream.  vB first (needed by AV of head 8 at
                    # iter 9), kT pairs 4-7 spliced to land before their
                    # scores (head 2p at iter 2p).
                    def vb_piece(tt):
                        def run():
                            psv = ps_x.tile([128, QR], F32, tag="x", name="psvb")
                            for c in range(NCT):
                                nc.tensor.matmul(
                                    psv[:],
                                    xT[:, c, tt * 128:(tt + 1) * 128],
                                    wv[:, c, QR:2 * QR],
                                    start=(c == 0), stop=(c == NCT - 1))
                            nc.vector.tensor_copy(
                                vext[:, tt, 8:16, 0:64],
                                psv[:].rearrange("p (h d) -> p h d", h=8))
                        return run

                    def kt_piece(p, u):
                        def run():
                            psk = ps_x.tile([128, QR], F32, tag="x", name="pskb")
                            for c in range(NCT):
                                nc.tensor.matmul(
                                    psk[:],
                                    wk[:, c, p * 128:(p + 1) * 128],
                                    xT[:, c, u * QR:(u + 1) * QR],
                                    start=(c == 0), stop=(c == NCT - 1))
                            nc.scalar.activation(kT[:, p, u * QR:(u + 1) * QR], psk[:], Copy)
                        return run

                    # per-iteration piece lists.  Deadlines: kT pair p must be
                    # fully written before iteration 2p's scores (slot <=
                    # 2p-1); vB before iteration 9's AV of head 8.
                    pieces = [[] for _ in range(H)]
                    pieces[0] = [vb_piece(0), vb_piece(1)]
                    pieces[1] = [vb_piece(2), vb_piece(3)]
                    pieces[2] = [vb_piece(4)]
                    pieces[3] = [vb_piece(5)]
                    pieces[4] = [vb_piece(6)]
                    pieces[5] = [vb_piece(7)]
                    pieces[6] = [kt_piece(4, 0)]
                    pieces[7] = [kt_piece(4, 1)]
                    pieces[8] = [kt_piece(5, 0)]
                    pieces[9] = [kt_piece(5, 1)]
                    pieces[10] = [kt_piece(6, 0)]
                    pieces[11] = [kt_piece(6, 1)]
                    pieces[12] = [kt_piece(7, 0)]
                    pieces[13] = [kt_piece(7, 1)]

                    tiles = {}  # h -> list of (level, at tile)
                    oTs = {}  # h -> oT psum
                    recbs = {}  # h -> broadcast 1/sumexp tile

                    def stage_s(h, js):
                        p, po = h // 2, (h % 2) * 64
                        lst = tiles.setdefault(h, [])
                        for j in js:
                            st = 128 * j
                            sps = ps_s.tile([128, 2, 512], F32, tag="s")
                            for sub in range(2):
                                kb = j + 4 * sub
                                nc.tensor.matmul(
                                    sps[:, sub, st:],
                                    kT[po:po + 64, p, kb * 128:(kb + 1) * 128],
                                    qT[po:po + 64, p, st:],
                                    start=True,
                                    stop=True,
                                )
                            at = attnp.tile([128, 2, 512], BF16, tag="at")
                            nc.scalar.activation(at[:, :, st:], sps[:, :, st:], Exp, scale=0.125)
                            eng = nc.vector if j % 2 == 0 else nc.gpsimd
                            eng.tensor_mul(
                                at[:, :, st:st + 128], at[:, :, st:st + 128], m2[:])
                            lst.append((j, at))

                    def stage_a(h):
                        oT = ps_o.tile([65, QR], F32, tag="o")
                        for j, at in tiles.pop(h):
                            st = 128 * j
                            for sub in range(2):
                                kb = j + 4 * sub
                                nc.tensor.matmul(
                                    oT[:, st:],
                                    vext[:, kb, h, :],
                                    at[:, sub, st:],
                                    start=(j == 0 and sub == 0),
                                    stop=(j == NL - 1 and sub == 1),
                                    skip_group_check=True,
                                )
                        rec = smalls.tile([1, QR], F32, tag="rec")
                        if h < H - 3:
                            nc.vector.reciprocal(rec[:], oT[64:65, :])
                        else:
                            with nc.allow_low_precision(reason="fp32r relabel of fp32 reciprocal"):
                                nc.vector.reciprocal(r(rec[:]), oT[64:65, :])
                        # copy oT out of PSUM immediately so the bank frees
                        # without waiting for the broadcast round trip
                        oTc = smalls.tile([64, QR], BF16, tag="oTc")
                        with nc.allow_low_precision(reason="attention output rounded to bf16"):
                            nc.vector.tensor_copy(oTc[:], oT[0:64, :])
                        oTs[h] = oTc
                        if h < H - 3:
                            # bounce through DRAM to broadcast across 64 rows
                            nc.sync.dma_start(recscr_d[h:h + 1, :], rec[:])
                            recb = smalls.tile([64, QR], F32, tag="recb")
                            nc.sync.dma_start(
                                recb[:], recscr_d[h:h + 1, :].partition_broadcast(64))
                            recbs[h] = recb
                        else:
                            # tail heads: PE rank-1 broadcast avoids the DMA
                            # round-trip latency on the critical path
                            recbs[h] = rec

                    def stage_n(h):
                        p, po = h // 2, (h % 2) * 64
                        oTc = oTs.pop(h)
                        recb = recbs.pop(h)
                        if h >= H - 3:
                            bcf = ps_x.tile([128, QR], F32, tag="x", name="bc")
                            nc.tensor.matmul(bcf[0:64, :], r(ones1f[:]), r(recb[:]),
                                             start=True, stop=True)
                            recb = bcf[0:64, :]
                        with nc.allow_low_precision(reason="attention output rounded to bf16"):
                            nc.vector.tensor_mul(proj_in[po:po + 64, p, :], oTc[:], recb[:])

                    for h in range(H):
                        stage_s(h, (0, 1))
                        for piece in pieces[h]:
                            piece()
                        stage_s(h, (2, 3))
                        if h >= 1:
                            stage_a(h - 1)
                        if h >= 2:
                            stage_n(h - 2)
                    # tight drain: the last head's AV/normalize gates the
                    # output projection and the final 2MB DMA
                    stage_a(H - 1)
                    stage_n(H - 2)
                    stage_n(H - 1)

            # ---------------- output projection ----------------
            with (
                tc.tile_pool(name="fin", bufs=2) as finp,
                tc.tile_pool(name="ps_f", bufs=1, space="PSUM") as ps_f,
            ):
                # all four row-blocks accumulate concurrently (8 banks); the
                # last head pair's contribution comes last so everything else
                # overlaps the attention drain.
                psfs = [
                    ps_f.tile([128, C], F32, tag=f"f{m}", name=f"psf{m}", bufs=1)
                    for m in range(QR // 128)
                ]
                for p in range(NP - 1):
                    for m in range(QR // 128):
                        psf = psfs[m]
                        lhs = proj_in[:, p, m * 128:(m + 1) * 128]
                        nc.tensor.matmul(psf[:, 0:512], lhs, woT[:, p, 0:512],
                                         start=(p == 0), stop=False,
                                         skip_group_check=True)
                        nc.tensor.matmul(psf[:, 512:1024], lhs, woT[:, p, 512:1024],
                                         start=(p == 0), stop=False,
                                         skip_group_check=True)
                for m in range(QR // 128):
                    psf = psfs[m]
                    lhs = proj_in[:, NP - 1, m * 128:(m + 1) * 128]
                    nc.tensor.matmul(psf[:, 0:512], lhs, woT[:, NP - 1, 0:512],
                                     start=False, stop=False, skip_group_check=True)
                    nc.tensor.matmul(psf[:, 512:1024], lhs, woT[:, NP - 1, 512:1024],
                                     start=False, stop=False, skip_group_check=True)
                    # bias via rank-1 accumulate: psf += 1 (x) bo
                    nc.tensor.matmul(psf[:, 0:512], wlhs[:], bo1[0:1, 0:512],
                                     start=False, stop=True, skip_group_check=True)
                    nc.tensor.matmul(psf[:, 512:1024], wlhs[:], bo1[0:1, 512:1024],
                                     start=False, stop=True, skip_group_check=True)
                    fin = finp.tile([128, C], F32, tag="fin")
                    # halves on alternating engines so the copies drain 2x
                    for u in range(2):
                        half = slice(u * 512, (u + 1) * 512)
                        if u == 0:
                            nc.scalar.activation(fin[:, half], psf[:, half], Copy)
                        else:
                            nc.vector.tensor_copy(fin[:, half], psf[:, half])
                        nc.sync.dma_start(
                            out_d[m * 128:(m + 1) * 128, half], fin[:, half])

    nc.compile()
    return nc


def get_nc():
    if "nc" not in _CACHE:
        _CACHE["nc"] = _build()
    return _CACHE["nc"]


def make_in_maps(x, Wq, Wk, Wv, Wo, bo):
    import ml_dtypes

    BF = ml_dtypes.bfloat16
    x = np.asarray(x, dtype=np.float32)
    wq = np.ascontiguousarray(
        np.asarray(Wq, np.float32).transpose(1, 0, 2).reshape(C, H * D).astype(BF))
    wk = np.ascontiguousarray(
        np.asarray(Wk, np.float32).transpose(1, 0, 2).reshape(C, H * D).astype(BF))
    wv = np.ascontiguousarray(
        np.asarray(Wv, np.float32).transpose(1, 0, 2).reshape(C, H * D).astype(BF))
    woT = np.ascontiguousarray(np.asarray(Wo, np.float32).T.astype(BF))
    bo1 = np.ascontiguousarray(np.asarray(bo, np.float32).reshape(1, C).astype(BF))
    k_ = np.arange(128)[:, None]
    i_ = np.arange(128)[None, :]
    m_own = (k_ <= i_).astype(BF)  # own-parity boundary block: k <= q
    in_maps = []
    for core in range(N_CORES):
        b, par = core // 2, core % 2
        perm = np.concatenate([np.arange(par, T, 2), np.arange(1 - par, T, 2)])
        xTp = np.ascontiguousarray(x[b].T[:, perm].astype(BF))
        # other-parity keys 2k+(1-par) vs queries 2i+par: strict for par=0
        m_oth = ((k_ < i_) if par == 0 else (k_ <= i_)).astype(BF)
        m2 = np.ascontiguousarray(np.stack([m_own, m_oth], axis=1))
        in_maps.append({
            "xT": xTp, "wq": wq, "wk": wk, "wv": wv,
            "woT": woT, "bo1": bo1, "m2": m2,
        })
    return in_maps


def kernel(x, Wq, Wk, Wv, Wo, bo):
    from concourse.bass_utils import run_bass_kernel_spmd

    nc = get_nc()
    in_maps = make_in_maps(x, Wq, Wk, Wv, Wo, bo)
    res = run_bass_kernel_spmd(nc, in_maps, list(range(N_CORES)))
    out = np.empty((B, T, C), np.float32)
    for core in range(N_CORES):
        b, par = core // 2, core % 2
        out[b, par::2, :] = res.results[core]["out"]
    return out
